# revision 31
# baseline (speedup 1.0000x reference)
"""Mean-pooling (segment mean over sorted segment ids) Trainium2 kernel.

Problem: x [1_000_000, 128] f32, batch [1_000_000] int64 (sorted, values in
[0, 8192)).  Outputs: graph_embedding [8192, 128] f32 (segment mean, count
clamped to 1) and attention_scores [1_000_000] f32 (1/count gathered per node).

Strategy (data-parallel over nodes, 8 cores, one uniform SPMD program):
 - Each core streams a contiguous ~125k-row slice of x (pre-cast to fp16 on
   the host; RMS output error ~2e-4, far inside the resid_var 1e-4 gate)
   as 32-chunk DMA slabs in a host-packed partition-major layout (one DMA
   per slab, multi-KiB descriptors, alternating the two HWDGE rings).
 - Segment sums are matmuls with narrow one-hot matrices N_j [128 nodes, C]
   built on the host from the sorted `batch`: column k of a chunk means
   graph g0+k, where g0 is the first graph of the chunk's 4-chunk
   "superchunk" (512 sorted nodes span only a handful of graphs).
 - The 4 chunks of a superchunk accumulate into the same [C, 128] PSUM
   region; 4 superchunks use the 4 disjoint 128-column sets of one PSUM
   bank [C, 512].  One accumulation group per bank (start on the first
   matmul, stop on the last) -- start=True clears the whole bank.
 - One VectorE copy [C, 512] evacuates each bank, then one DMA (on the
   opposite HWDGE ring from the slab loads, avoiding head-of-line blocks)
   writes the partials out (16 KiB per 2 MiB of x read).
 - The host scatter-adds the ~4k partial rows per core into the [8192, 128]
   sum (column -> graph mapping is known on the host), divides by bincount,
   and gathers the attention scores.  All heavy traffic (the 256 MB of
   fp16 x) stays on the device.
"""

import math

import numpy as np

P = 128          # partitions / nodes per chunk
D = 128          # hidden dim
SUPER = 4        # chunks per superchunk (shared one-hot column base)
SLAB = 16        # chunks per PSUM bank / output group
DSLAB = 32       # chunks per x DMA (bigger descriptors)

NCORES = 8
N_NODES = 1_000_000
G_TOTAL = 8192
CHUNKS = 977                    # chunks per core
SHARD_ROWS = CHUNKS * P         # 125_056
S_STRIDE = 124_992              # row stride between consecutive cores' shards
OVL = 64                        # overlap rows between consecutive shards
C_MAIN = 8                      # one-hot columns (max graph span per superchunk)

_PROGRAM_CACHE = {}


def _build_program(n_chunks, C, repeat=1, mode="split"):
    """Uniform per-core Bass program: segment-sum partials for one shard.

    mode:
      "f32"   - x stays fp32 (PE streams fp32 at ~4 cycles/col: slow, 5e-7)
      "f32r"  - fp32 data streamed in single-pass float32r mode (PE 4x
                faster than fp32, ~1e-4 error, no host-side conversion)
      "fp16"  - x pre-cast to fp16 on the host (fast, ~2e-4 error; default)
      "bf16"  - x pre-cast to bf16 on the host (fast, ~2e-3 error)
      "split" - x as bf16 hi + bf16 lo residual, both accumulated into the
                same PSUM region (error-compensated: ~1e-6, PE 2x faster
                than fp32, same DMA volume)
    repeat>1 re-runs the whole body (for overhead-free device timing).
    """
    import concourse.bacc as bacc
    import concourse.tile as tile
    from concourse import mybir

    f32 = mybir.dt.float32
    bf16 = mybir.dt.bfloat16
    xdt = {
        "f32": f32,
        "f32r": mybir.dt.float32r,
        "fp16": mybir.dt.float16,
    }.get(mode, bf16)
    n_streams = 2 if mode == "split" else 1
    n_banks = math.ceil(n_chunks / SLAB)
    n_dma = math.ceil(n_chunks / DSLAB)
    # host packs per DMA-slab: [P, n_streams*DSLAB*D + DSLAB*C] (x in
    # partition-major layout + the one-hot block): ONE DMA per slab with
    # multi-KiB descriptors
    XW = DSLAB * D
    ROW = n_streams * XW + DSLAB * C

    nc = bacc.Bacc("TRN2", target_bir_lowering=False, debug=False)
    xin = nc.dram_tensor(
        "xin", [n_dma, P, ROW], xdt, kind="ExternalInput"
    ).ap()
    outp = nc.dram_tensor(
        "partials", [n_banks, C, SUPER, D], f32, kind="ExternalOutput"
    ).ap()

    with tile.TileContext(nc) as tc:
        with (
            tc.tile_pool(name="xp", bufs=6) as xp,
            tc.tile_pool(name="pp", bufs=8, space="PSUM") as pp,
            tc.tile_pool(name="op", bufs=8) as op,
        ):
            for _ in range(repeat):
                for s in range(n_dma):
                    ch = min(DSLAB, n_chunks - s * DSLAB)
                    xt = xp.tile([P, ROW], xdt, tag="xt")
                    eng = nc.sync if s % 2 == 0 else nc.scalar
                    eng.dma_start(out=xt[:], in_=xin[s])
                    for b in range(math.ceil(ch / SLAB)):
                        bch = min(SLAB, ch - b * SLAB)
                        ps = pp.tile([C, SUPER * D], f32, tag="ps")
                        n_mm = bch * n_streams
                        mm = 0
                        for k in range(bch):
                            cj = b * SLAB + k       # chunk within DMA slab
                            m = k // SUPER
                            for i in range(n_streams):
                                nc.tensor.matmul(
                                    out=ps[:, m * D : (m + 1) * D],
                                    lhsT=xt[
                                        :,
                                        n_streams * XW + cj * C
                                        : n_streams * XW + (cj + 1) * C,
                                    ],
                                    rhs=xt[
                                        :,
                                        i * XW + cj * D
                                        : i * XW + (cj + 1) * D,
                                    ],
                                    start=(mm == 0),
                                    stop=(mm == n_mm - 1),
                                )
                                mm += 1
                        ot = op.tile([C, SUPER * D], f32, tag="ot")
                        nc.vector.tensor_copy(out=ot[:], in_=ps[:])
                        oeng = nc.scalar if s % 2 == 0 else nc.sync
                        oeng.dma_start(out=outp[s * (DSLAB // SLAB) + b],
                                       in_=ot[:])
    nc.compile()
    return nc


def _build_host_side(batch, n_chunks, C, shard_starts, own_bounds):
    """Per-core one-hot tensors + column->graph maps from sorted batch ids.

    Returns nmats [n_dma, P, DSLAB*C] f32 and col_graph [n_supers, C] int64
    (-1 marks invalid columns) per core.
    """
    n_dma = math.ceil(n_chunks / DSLAB)
    n_supers = math.ceil(n_chunks / SLAB) * SUPER
    nmats, col_graphs = [], []
    rows_per_super = SUPER * P
    for c, start in enumerate(shard_starts):
        shard = np.asarray(batch[start : start + n_chunks * P])
        nrows = shard.shape[0]
        row = np.arange(nrows)
        sup = row // rows_per_super                      # superchunk index
        g0 = shard[sup * rows_per_super]                 # base graph per row
        off = shard - g0
        if off.max() >= C:
            raise OverflowError("superchunk spans more than C graphs")
        own_lo, own_hi = own_bounds[c]
        owned = ((start + row) >= own_lo) & ((start + row) < own_hi)
        j = row // P                                     # chunk index
        p = row % P
        nm = np.zeros((n_dma, P, DSLAB * C), np.float32)
        nm[j // DSLAB, p, (j % DSLAB) * C + off] = owned.astype(np.float32)

        sup_starts = np.arange(n_supers) * rows_per_super
        sup_starts = np.minimum(sup_starts, nrows - 1)
        sup_ends = np.minimum(sup_starts + rows_per_super, nrows) - 1
        g_lo = shard[sup_starts]
        g_hi = shard[sup_ends]
        ks = np.arange(C)
        cg = g_lo[:, None] + ks[None, :]
        cg[cg > g_hi[:, None]] = -1
        # superchunks past the end of real data are invalid
        n_real_supers = math.ceil(n_chunks / SUPER)
        cg[n_real_supers:] = -1
        nmats.append(nm)
        col_graphs.append(cg.astype(np.int64))
    return nmats, col_graphs


def _run(x, batch, n_chunks, C, shard_starts, own_bounds, num_graphs,
         trace=False, mode="split"):
    import ml_dtypes
    from concourse.bass_utils import run_bass_kernel_spmd

    bf16 = ml_dtypes.bfloat16
    key = (n_chunks, C, mode)
    if key not in _PROGRAM_CACHE:
        _PROGRAM_CACHE[key] = _build_program(n_chunks, C, mode=mode)
    nc = _PROGRAM_CACHE[key]

    nmats, col_graphs = _build_host_side(
        batch, n_chunks, C, shard_starts, own_bounds
    )
    x = np.asarray(x)
    if mode in ("f32", "f32r"):
        streams, ndt = [x], np.float32
    elif mode == "fp16":
        streams, ndt = [x.astype(np.float16)], np.float16
    elif mode == "bf16":
        streams, ndt = [x.astype(bf16)], bf16
    else:
        xh = x.astype(bf16)
        xl = (x - xh.astype(np.float32)).astype(bf16)
        streams, ndt = [xh, xl], bf16

    n_dma = math.ceil(n_chunks / DSLAB)
    rows_pad = n_dma * DSLAB * P

    def pack_shard(start, nmat):
        blocks = []
        for st in streams:
            sh = np.zeros((rows_pad, D), ndt)
            sh[: n_chunks * P] = st[start : start + n_chunks * P]
            blocks.append(
                sh.reshape(n_dma, DSLAB, P, D)
                .transpose(0, 2, 1, 3)
                .reshape(n_dma, P, DSLAB * D)
            )
        blocks.append(nmat.astype(ndt))
        return np.ascontiguousarray(np.concatenate(blocks, axis=2))

    in_maps = [
        {"xin": pack_shard(start, nmats[c])}
        for c, start in enumerate(shard_starts)
    ]
    ncores = len(shard_starts)
    res = run_bass_kernel_spmd(
        nc, in_maps, core_ids=list(range(ncores)), trace=trace
    )

    seg = np.zeros((num_graphs, D), np.float32)
    for c in range(ncores):
        # partials[s, k, m, d] -> row for super (s*SUPER + m), column k
        partials = (
            res.results[c]["partials"]
            .transpose(0, 2, 1, 3)
            .reshape(-1, C, D)
        )
        cg = col_graphs[c]
        mask = cg >= 0
        np.add.at(seg, cg[mask], partials[mask])
    return seg, res


def kernel(x, batch):
    x = np.asarray(x)
    batch = np.asarray(batch)
    assert x.shape == (N_NODES, D) and batch.shape == (N_NODES,)

    counts = np.bincount(batch, minlength=G_TOTAL).astype(np.float32)
    denom = np.maximum(counts, 1.0)

    shard_starts = [S_STRIDE * c for c in range(NCORES)]
    own_bounds = []
    for c in range(NCORES):
        lo = 0 if c == 0 else S_STRIDE * c + OVL
        hi = S_STRIDE * (c + 1) + OVL if c < NCORES - 1 else N_NODES
        own_bounds.append((lo, hi))

    try:
        seg, _ = _run(x, batch, CHUNKS, C_MAIN, shard_starts, own_bounds,
                      G_TOTAL, mode="fp16")
    except Exception:
        # safety net: inputs whose 512-node windows span > C_MAIN graphs
        # (OverflowError) or any transient device failure -> exact host path
        seg = np.zeros((G_TOTAL, D), np.float32)
        np.add.at(seg, batch, x)

    graph_embedding = seg / denom[:, None]
    attention_scores = (1.0 / denom).astype(np.float32)[batch]
    return graph_embedding, attention_scores


# revision 35
# speedup vs baseline: 1.0360x; 1.0360x over previous
"""Mean-pooling (segment mean over sorted segment ids) Trainium2 kernel.

Problem: x [1_000_000, 128] f32, batch [1_000_000] int64 (sorted, values in
[0, 8192)).  Outputs: graph_embedding [8192, 128] f32 (segment mean, count
clamped to 1) and attention_scores [1_000_000] f32 (1/count gathered per node).

Strategy (data-parallel over nodes, 8 cores, one uniform SPMD program):
 - Each core streams a contiguous ~125k-row slice of x (pre-cast to fp16 on
   the host; RMS output error ~2e-4, far inside the resid_var 1e-4 gate)
   as 32-chunk DMA slabs in a host-packed partition-major layout (one DMA
   per slab, multi-KiB descriptors, alternating the two HWDGE rings).
 - Segment sums are matmuls with narrow one-hot matrices N_j [128 nodes, C]
   built on the host from the sorted `batch`: column k of a chunk means
   graph g0+k, where g0 is the first graph of the chunk's 4-chunk
   "superchunk" (512 sorted nodes span only a handful of graphs).
 - The 4 chunks of a superchunk accumulate into the same [C, 128] PSUM
   region; 4 superchunks use the 4 disjoint 128-column sets of one PSUM
   bank [C, 512].  One accumulation group per bank (start on the first
   matmul, stop on the last) -- start=True clears the whole bank.
 - One VectorE copy [C, 512] evacuates each bank, then one DMA (on the
   opposite HWDGE ring from the slab loads, avoiding head-of-line blocks)
   writes the partials out (16 KiB per 2 MiB of x read).
 - The host scatter-adds the ~4k partial rows per core into the [8192, 128]
   sum (column -> graph mapping is known on the host), divides by bincount,
   and gathers the attention scores.  All heavy traffic (the 256 MB of
   fp16 x) stays on the device.
"""

import math

import numpy as np

P = 128          # partitions / nodes per chunk
D = 128          # hidden dim
SUPER = 4        # chunks per superchunk (shared one-hot column base)
SLAB = 16        # chunks per PSUM bank / output group
DSLAB = 32       # chunks per x DMA (bigger descriptors)

NCORES = 8
N_NODES = 1_000_000
G_TOTAL = 8192
CHUNKS = 977                    # chunks per core
SHARD_ROWS = CHUNKS * P         # 125_056
S_STRIDE = 124_992              # row stride between consecutive cores' shards
OVL = 64                        # overlap rows between consecutive shards
C_MAIN = 8                      # one-hot columns (max graph span per superchunk)

_PROGRAM_CACHE = {}


def _build_program(n_chunks, C, repeat=1, mode="split"):
    """Uniform per-core Bass program: segment-sum partials for one shard.

    mode:
      "f32"   - x stays fp32 (PE streams fp32 at ~4 cycles/col: slow, 5e-7)
      "f32r"  - fp32 data streamed in single-pass float32r mode (PE 4x
                faster than fp32, ~1e-4 error, no host-side conversion)
      "fp16"  - x pre-cast to fp16 on the host (fast, ~2e-4 error; default)
      "bf16"  - x pre-cast to bf16 on the host (fast, ~2e-3 error)
      "split" - x as bf16 hi + bf16 lo residual, both accumulated into the
                same PSUM region (error-compensated: ~1e-6, PE 2x faster
                than fp32, same DMA volume)
    repeat>1 re-runs the whole body (for overhead-free device timing).
    """
    import concourse.bacc as bacc
    import concourse.tile as tile
    from concourse import mybir

    f32 = mybir.dt.float32
    bf16 = mybir.dt.bfloat16
    xdt = {
        "f32": f32,
        "f32r": mybir.dt.float32r,
        "fp16": mybir.dt.float16,
    }.get(mode, bf16)
    n_streams = 2 if mode == "split" else 1
    n_banks = math.ceil(n_chunks / SLAB)
    n_dma = math.ceil(n_chunks / DSLAB)
    # host packs per DMA-slab: [P, n_streams*DSLAB*D + one-hot block] (x in
    # partition-major layout): ONE DMA per slab with multi-KiB descriptors.
    # In fp16 mode the one-hot block is stored as fp8 bytes (0/1 exact)
    # inside the fp16 tensor and bitcast on device: half the bytes.
    fp8_onehot = mode == "fp16"
    XW = DSLAB * D
    NW = DSLAB * C // 2 if fp8_onehot else DSLAB * C
    ROW = n_streams * XW + NW

    nc = bacc.Bacc("TRN2", target_bir_lowering=False, debug=False)
    xin = nc.dram_tensor(
        "xin", [n_dma, P, ROW], xdt, kind="ExternalInput"
    ).ap()
    outp = nc.dram_tensor(
        "partials", [n_banks, C, SUPER, D], f32, kind="ExternalOutput"
    ).ap()

    with tile.TileContext(nc) as tc:
        with (
            tc.tile_pool(name="xp", bufs=6) as xp,
            tc.tile_pool(name="pp", bufs=8, space="PSUM") as pp,
            tc.tile_pool(name="op", bufs=8) as op,
        ):
            for _ in range(repeat):
                for s in range(n_dma):
                    ch = min(DSLAB, n_chunks - s * DSLAB)
                    xt = xp.tile([P, ROW], xdt, tag="xt")
                    eng = nc.sync if s % 2 == 0 else nc.scalar
                    eng.dma_start(out=xt[:], in_=xin[s])
                    if fp8_onehot:
                        nt = xt[:, n_streams * XW : ROW].bitcast(
                            mybir.dt.float8e4
                        )
                    else:
                        nt = xt[:, n_streams * XW : ROW]
                    for b in range(math.ceil(ch / SLAB)):
                        bch = min(SLAB, ch - b * SLAB)
                        ps = pp.tile([C, SUPER * D], f32, tag="ps")
                        n_mm = bch * n_streams
                        mm = 0
                        for k in range(bch):
                            cj = b * SLAB + k       # chunk within DMA slab
                            m = k // SUPER
                            for i in range(n_streams):
                                nc.tensor.matmul(
                                    out=ps[:, m * D : (m + 1) * D],
                                    lhsT=nt[:, cj * C : (cj + 1) * C],
                                    rhs=xt[
                                        :,
                                        i * XW + cj * D
                                        : i * XW + (cj + 1) * D,
                                    ],
                                    start=(mm == 0),
                                    stop=(mm == n_mm - 1),
                                )
                                mm += 1
                        ot = op.tile([C, SUPER * D], f32, tag="ot")
                        nc.vector.tensor_copy(out=ot[:], in_=ps[:])
                        oeng = nc.scalar if s % 2 == 0 else nc.sync
                        oeng.dma_start(out=outp[s * (DSLAB // SLAB) + b],
                                       in_=ot[:])
    nc.compile()
    return nc


def _build_host_side(batch, n_chunks, C, shard_starts, own_bounds):
    """Per-core one-hot tensors + column->graph maps from sorted batch ids.

    Returns nmats [n_dma, P, DSLAB*C] f32 and col_graph [n_supers, C] int64
    (-1 marks invalid columns) per core.
    """
    n_dma = math.ceil(n_chunks / DSLAB)
    n_supers = math.ceil(n_chunks / SLAB) * SUPER
    nmats, col_graphs = [], []
    rows_per_super = SUPER * P
    for c, start in enumerate(shard_starts):
        shard = np.asarray(batch[start : start + n_chunks * P])
        nrows = shard.shape[0]
        row = np.arange(nrows)
        sup = row // rows_per_super                      # superchunk index
        g0 = shard[sup * rows_per_super]                 # base graph per row
        off = shard - g0
        if off.max() >= C:
            raise OverflowError("superchunk spans more than C graphs")
        own_lo, own_hi = own_bounds[c]
        owned = ((start + row) >= own_lo) & ((start + row) < own_hi)
        j = row // P                                     # chunk index
        p = row % P
        nm = np.zeros((n_dma, P, DSLAB * C), np.float32)
        nm[j // DSLAB, p, (j % DSLAB) * C + off] = owned.astype(np.float32)

        sup_starts = np.arange(n_supers) * rows_per_super
        sup_starts = np.minimum(sup_starts, nrows - 1)
        sup_ends = np.minimum(sup_starts + rows_per_super, nrows) - 1
        g_lo = shard[sup_starts]
        g_hi = shard[sup_ends]
        ks = np.arange(C)
        cg = g_lo[:, None] + ks[None, :]
        cg[cg > g_hi[:, None]] = -1
        # superchunks past the end of real data are invalid
        n_real_supers = math.ceil(n_chunks / SUPER)
        cg[n_real_supers:] = -1
        nmats.append(nm)
        col_graphs.append(cg.astype(np.int64))
    return nmats, col_graphs


def _run(x, batch, n_chunks, C, shard_starts, own_bounds, num_graphs,
         trace=False, mode="split"):
    import ml_dtypes
    from concourse.bass_utils import run_bass_kernel_spmd

    bf16 = ml_dtypes.bfloat16
    key = (n_chunks, C, mode)
    if key not in _PROGRAM_CACHE:
        _PROGRAM_CACHE[key] = _build_program(n_chunks, C, mode=mode)
    nc = _PROGRAM_CACHE[key]

    nmats, col_graphs = _build_host_side(
        batch, n_chunks, C, shard_starts, own_bounds
    )
    x = np.asarray(x)
    if mode in ("f32", "f32r"):
        streams, ndt = [x], np.float32
    elif mode == "fp16":
        streams, ndt = [x.astype(np.float16)], np.float16
    elif mode == "bf16":
        streams, ndt = [x.astype(bf16)], bf16
    else:
        xh = x.astype(bf16)
        xl = (x - xh.astype(np.float32)).astype(bf16)
        streams, ndt = [xh, xl], bf16

    n_dma = math.ceil(n_chunks / DSLAB)
    rows_pad = n_dma * DSLAB * P

    def pack_shard(start, nmat):
        blocks = []
        for st in streams:
            sh = np.zeros((rows_pad, D), ndt)
            sh[: n_chunks * P] = st[start : start + n_chunks * P]
            blocks.append(
                sh.reshape(n_dma, DSLAB, P, D)
                .transpose(0, 2, 1, 3)
                .reshape(n_dma, P, DSLAB * D)
            )
        if mode == "fp16":
            # one-hot stored as fp8e4m3 bytes (0/1 exact), viewed as fp16
            # so it rides in the same tensor; device bitcasts it back
            nm8 = nmat.astype(ml_dtypes.float8_e4m3)
            blocks.append(nm8.view(np.uint8).view(np.float16))
        else:
            blocks.append(nmat.astype(ndt))
        return np.ascontiguousarray(np.concatenate(blocks, axis=2))

    in_maps = [
        {"xin": pack_shard(start, nmats[c])}
        for c, start in enumerate(shard_starts)
    ]
    ncores = len(shard_starts)
    res = run_bass_kernel_spmd(
        nc, in_maps, core_ids=list(range(ncores)), trace=trace
    )

    seg = np.zeros((num_graphs, D), np.float32)
    for c in range(ncores):
        # partials[s, k, m, d] -> row for super (s*SUPER + m), column k
        partials = (
            res.results[c]["partials"]
            .transpose(0, 2, 1, 3)
            .reshape(-1, C, D)
        )
        cg = col_graphs[c]
        mask = cg >= 0
        np.add.at(seg, cg[mask], partials[mask])
    return seg, res


def kernel(x, batch):
    x = np.asarray(x)
    batch = np.asarray(batch)
    assert x.shape == (N_NODES, D) and batch.shape == (N_NODES,)

    counts = np.bincount(batch, minlength=G_TOTAL).astype(np.float32)
    denom = np.maximum(counts, 1.0)

    shard_starts = [S_STRIDE * c for c in range(NCORES)]
    own_bounds = []
    for c in range(NCORES):
        lo = 0 if c == 0 else S_STRIDE * c + OVL
        hi = S_STRIDE * (c + 1) + OVL if c < NCORES - 1 else N_NODES
        own_bounds.append((lo, hi))

    try:
        seg, _ = _run(x, batch, CHUNKS, C_MAIN, shard_starts, own_bounds,
                      G_TOTAL, mode="fp16")
    except Exception:
        # safety net: inputs whose 512-node windows span > C_MAIN graphs
        # (OverflowError) or any transient device failure -> exact host path
        seg = np.zeros((G_TOTAL, D), np.float32)
        np.add.at(seg, batch, x)

    graph_embedding = seg / denom[:, None]
    attention_scores = (1.0 / denom).astype(np.float32)[batch]
    return graph_embedding, attention_scores
